# revision 62
# baseline (speedup 1.0000x reference)
"""Trainium2 Bass kernel for nn_Attention4D_77644418777285.

Attention4D block (EfficientViT-style): 1x1-conv QKV + BN, depthwise-3x3
local-V branch, relative-position bias, talking-heads attention (8 heads,
49 tokens), projection. Batch 512 sharded 64-per-core across 8 NeuronCores
(pure data parallel; weights replicated).

Strategy (per core, 64 images = 3136 tokens):
  - x transposed on PE to channel-major xT [384, 3136] (bf16).
  - QKV as channel-major matmuls (weights stationary), BN folded into
    weights/biases on host; softmax scale folded into q weights.
  - Attention middle processed in groups of 8 images with logits held as
    [(head-pair, m) x (img, n)] tiles: per-(img,head) qk matmuls, talking
    heads th1/th2 as constant 128x128 block matmuls, softmax (exp on ACT,
    column sums via a constant selector matmul, reciprocal on DVE,
    normalization broadcast via constant delta matmul).
  - v also computed token-major per image pair (separate matmul) for the
    attention*V product; output is channel-major o^T per head.
  - depthwise 3x3 conv on DVE: 9 fused scalar_tensor_tensor taps over a
    zero-padded 9x9 channel-major layout; per-channel tap weights native as
    [P,1] scalars. All conv/BN biases folded into a precomputed B2 term
    that seeds the accumulator via DMA.
  - o^T + v_local + relu, projection back to channel-major, PE transpose to
    token-major f32 output.
"""

import numpy as np
import ml_dtypes

R = 7
N = 49
H = 8
KD = 32
D = 128
DH = 1024
DIM = 384
SCALE = KD ** -0.5
NCORES = 8
B_FULL = 512

CONV_DVE = 4       # chtiles 0..CONV_DVE-1 on DVE (TSP), rest fused on PE

_BF16 = ml_dtypes.bfloat16


def _bias_idxs(r):
    pos = np.stack(np.meshgrid(np.arange(r), np.arange(r))).reshape(2, -1)
    rel = np.abs(pos[:, :, None] - pos[:, None, :])
    return (rel[0] * r + rel[1]).reshape(-1)


def _host_consts(inp):
    """All weight-shaped tensors precomputed on host (numpy)."""
    f32 = np.float32
    g = {k: np.asarray(v, f32) for k, v in inp.items()}

    th1, th1_b = g['th1_w'], g['th1_b']
    th2, th2_b = g['th2_w'], g['th2_b']

    W_q = g['q_w'] * g['q_g'][None, :] * SCALE
    b_q = (g['q_b'] * g['q_g'] + g['q_beta']) * SCALE
    W_k = g['k_w'] * g['k_g'][None, :]
    b_k = g['k_b'] * g['k_g'] + g['k_beta']
    W_v = g['v_w'] * g['v_g'][None, :]
    b_v = g['v_b'] * g['v_g'] + g['v_beta']

    idxs = _bias_idxs(R)
    bias_full = g['attn_bias'][:, idxs].reshape(H, N, N)          # [h, n, m]
    biasp = np.einsum('hg,hnm->gnm', th1, bias_full) + th1_b[:, None, None]

    w9 = g['vl_w'].reshape(9, DH)                                  # [tap, c]
    w_eff = (w9 * g['vl_g'][None, :]).astype(f32)                  # [tap, c]
    sumw = np.zeros((DH, N), f32)
    for t in range(9):
        dy, dx = t // 3 - 1, t % 3 - 1
        for s in range(N):
            y, x = s // 7, s % 7
            if 0 <= y + dy < 7 and 0 <= x + dx < 7:
                sumw[:, s] += w9[t]
    s2 = th2.sum(axis=0) + N * th2_b                               # [g]
    B2 = (g['vl_g'][:, None] * (b_v[:, None] * sumw + g['vl_b'][:, None])
          + g['vl_beta'][:, None]
          + (b_v * s2[np.repeat(np.arange(H), D)])[:, None])       # [c, s=49]

    W_p = g['proj_w'] * g['proj_g'][None, :]
    b_p = g['proj_b'] * g['proj_g'] + g['proj_beta']

    consts = {}
    # QKV weights: [3 ktiles, 128, 512] (q|k) and [3, 128, 1024] (v)
    wqk = np.concatenate([W_q, W_k], axis=1).reshape(3, 128, 512)
    consts['wqk'] = wqk.astype(_BF16)
    consts['wv'] = W_v.reshape(3, 128, DH).astype(_BF16)
    consts['wp'] = W_p.reshape(8, 128, DIM).astype(_BF16)
    consts['bqk'] = np.concatenate([b_q, b_k]).reshape(4, 128).astype(f32)
    consts['bp'] = b_p.reshape(3, 128).astype(f32)

    # Talking heads as [jo, ji, K=128, M=128] block matrices in the 0/64
    # partition layout: row (hh*64 + m) of input tile ji = head (2*ji+hh),
    # key m; col (hh*64 + m) of output tile jo = head (2*jo+hh), key m.
    def th_blocks(thw):
        Wb = np.zeros((4, 4, 128, 128), f32)
        for jo in range(4):
            for ji in range(4):
                for hhi in range(2):
                    for hho in range(2):
                        c = thw[2 * ji + hhi, 2 * jo + hho]
                        Wb[jo, ji, hhi * 64:(hhi + 1) * 64,
                           hho * 64:(hho + 1) * 64] += c * np.eye(64, dtype=f32)
        return Wb
    consts['w1s'] = th_blocks(th1).astype(_BF16)
    consts['w2s'] = th_blocks(th2).astype(_BF16)

    # m-dim lives in 64-slot layout (8y x 8x, pads zero): slot s=(y*8+x)
    vs = np.array([y * 8 + x for y in range(7) for x in range(7)])  # valid s
    sel = np.zeros((128, 2), f32)
    sel[vs, 0] = 1.0
    sel[64 + vs, 1] = 1.0
    consts['sel'] = sel.astype(_BF16)

    dlt = np.zeros((128, 128), f32)
    for j in range(4):
        dlt[32 * j + 0, 0:64] = 1.0
        dlt[32 * j + 1, 64:128] = 1.0
    consts['dlt'] = dlt.astype(_BF16)

    # bias' folded into the th1 psum accumulation as one extra matmul:
    # out[(hh*64+m), (i,n)] += bT[n, hh*64+m] via indicator rhs [n, (i,n')].
    bT = np.zeros((4, 64, 128), f32)
    for j in range(4):
        for hh in range(2):
            bT[j, 0:N, hh * 64 + vs] = biasp[2 * j + hh]           # [n, m-slot]
    consts['bT'] = bT.astype(_BF16)
    ind = np.zeros((64, 8 * N), f32)
    for i in range(8):
        ind[0:N, i * N:(i + 1) * N] = np.eye(N, dtype=f32)
    consts['ind'] = ind.astype(_BF16)

    # depthwise tap weights: sbuf [128, 8*9] (c-part, (ct, tap))
    # w_eff[tap, c]: c = ct*128 + p -> dst [p, ct, tap]
    w9t = w_eff.reshape(9, 8, 128).transpose(2, 1, 0).copy()       # [128, 8, 9]
    consts['w9t'] = w9t.astype(f32)

    # PE-path conv: diag tap matrices [8ct, 9, 128, 128] and B2^T [56, 8*128]
    dg = np.zeros((8, 9, 128, 128), f32)
    for ct in range(8):
        for t in range(9):
            np.fill_diagonal(dg[ct, t], w_eff[t, ct * 128:(ct + 1) * 128])
    consts['dg'] = dg.astype(_BF16)

    # B2 in 64-slot layout [8 chtiles, 128, 8y*8x] (pad slots zero)
    b2d = np.zeros((8, 128, 64), f32)
    b2d[:, :, vs] = B2.reshape(8, 128, N)
    consts['b2d'] = b2d.astype(_BF16)
    # b2T[s, ct*128+c] = b2d[ct, c, s]  (slot s on partitions)
    consts['b2T'] = np.ascontiguousarray(
        b2d.transpose(2, 0, 1).reshape(64, 1024)).astype(_BF16)
    # ind64[k, img*64+s] = (k == s), 8 image slots
    i64 = np.zeros((64, 512), f32)
    for q in range(8):
        i64[:, q * 64:(q + 1) * 64] = np.eye(64, dtype=f32)
    consts['ind64'] = i64.astype(_BF16)

    consts['ident'] = np.eye(128, dtype=f32).astype(_BF16)
    return consts


def build_program(n_imgs, stage=11):
    """Build the Bass program for one core processing n_imgs images."""
    from contextlib import ExitStack
    import concourse.bass as bass
    import concourse.tile as tile
    from concourse import bacc, mybir

    f32 = mybir.dt.float32
    bf16 = mybir.dt.bfloat16
    AF = mybir.ActivationFunctionType
    ALU = mybir.AluOpType

    NI = n_imgs
    NG = NI // 8                 # groups of 8 images
    NT = NI * N                  # tokens
    NTT = (NT + 127) // 128      # token tiles

    nc = bacc.Bacc("TRN2", target_bir_lowering=False, debug=False,
                   enable_asserts=False)

    x_d = nc.dram_tensor("x", [NT, DIM], bf16, kind="ExternalInput").ap()
    wqk_d = nc.dram_tensor("wqk", [3, 128, 512], bf16, kind="ExternalInput").ap()
    wv_d = nc.dram_tensor("wv", [3, 128, DH], bf16, kind="ExternalInput").ap()
    wp_d = nc.dram_tensor("wp", [8, 128, DIM], bf16, kind="ExternalInput").ap()
    bqk_d = nc.dram_tensor("bqk", [4, 128], f32, kind="ExternalInput").ap()
    bp_d = nc.dram_tensor("bp", [3, 128], f32, kind="ExternalInput").ap()
    w1_d = nc.dram_tensor("w1s", [4, 4, 128, 128], bf16, kind="ExternalInput").ap()
    w2_d = nc.dram_tensor("w2s", [4, 4, 128, 128], bf16, kind="ExternalInput").ap()
    sel_d = nc.dram_tensor("sel", [128, 2], bf16, kind="ExternalInput").ap()
    dlt_d = nc.dram_tensor("dlt", [128, 128], bf16, kind="ExternalInput").ap()
    bT_d = nc.dram_tensor("bT", [4, 64, 128], bf16, kind="ExternalInput").ap()
    ind_d = nc.dram_tensor("ind", [64, 392], bf16, kind="ExternalInput").ap()
    w9_d = nc.dram_tensor("w9t", [128, 8, 9], f32, kind="ExternalInput").ap()
    b2_d = nc.dram_tensor("b2d", [8, 128, 64], bf16, kind="ExternalInput").ap()
    dg_d = nc.dram_tensor("dg", [8, 9, 128, 128], bf16, kind="ExternalInput").ap()
    b2T_d = nc.dram_tensor("b2T", [64, 1024], bf16, kind="ExternalInput").ap()
    i64_d = nc.dram_tensor("ind64", [64, 512], bf16, kind="ExternalInput").ap()
    id_d = nc.dram_tensor("ident", [128, 128], bf16, kind="ExternalInput").ap()
    out_d = nc.dram_tensor("out", [NT, DIM], f32, kind="ExternalOutput").ap()

    with tile.TileContext(nc) as tc, ExitStack() as ctx:
        const = ctx.enter_context(tc.tile_pool(name="const", bufs=1))
        pers = ctx.enter_context(tc.tile_pool(name="pers", bufs=1))
        xin = ctx.enter_context(tc.tile_pool(name="xin", bufs=2))
        mid = ctx.enter_context(tc.tile_pool(name="mid", bufs=6))
        accp = ctx.enter_context(tc.tile_pool(name="accp", bufs=1))
        stg = ctx.enter_context(tc.tile_pool(name="stg", bufs=2))
        ps = ctx.enter_context(tc.tile_pool(name="ps", bufs=7, space="PSUM"))

        dma = nc.sync.dma_start

        # ---------------- constants ----------------
        wqk_t = [const.tile([128, 512], bf16, name=f"wqk{k}", tag=f"wqk{k}") for k in range(3)]
        wv_t = [const.tile([128, DH], bf16, name=f"wv{k}", tag=f"wv{k}") for k in range(3)]
        wp_t = [const.tile([128, DIM], bf16, name=f"wp{k}", tag=f"wp{k}") for k in range(8)]
        for k in range(3):
            dma(out=wqk_t[k], in_=wqk_d[k])
            dma(out=wv_t[k], in_=wv_d[k])
        for k in range(8):
            dma(out=wp_t[k], in_=wp_d[k])
        bqk_t = const.tile([128, 4], f32, name="bqk", tag="bqk")
        dma(out=bqk_t, in_=bass.AP(tensor=bqk_d.tensor, offset=0,
                                   ap=[[1, 128], [128, 4]]))
        bp_t = const.tile([128, 3], f32, name="bp", tag="bp")
        dma(out=bp_t, in_=bass.AP(tensor=bp_d.tensor, offset=0,
                                  ap=[[1, 128], [128, 3]]))
        w1_t = const.tile([128, 16, 128], bf16, name="w1", tag="w1")
        dma(out=w1_t, in_=bass.AP(tensor=w1_d.tensor, offset=0,
                                  ap=[[128, 128], [128 * 128, 16], [1, 128]]))
        w2_t = const.tile([128, 16, 128], bf16, name="w2", tag="w2")
        dma(out=w2_t, in_=bass.AP(tensor=w2_d.tensor, offset=0,
                                  ap=[[128, 128], [128 * 128, 16], [1, 128]]))
        sel_t = const.tile([128, 2], bf16, name="sel", tag="sel")
        dma(out=sel_t, in_=sel_d)
        dlt_t = const.tile([128, 128], bf16, name="dlt", tag="dlt")
        dma(out=dlt_t, in_=dlt_d)
        bT_t = const.tile([64, 4, 128], bf16, name="bT", tag="bT")
        dma(out=bT_t, in_=bass.AP(tensor=bT_d.tensor, offset=0,
                                  ap=[[128, 64], [64 * 128, 4], [1, 128]]))
        ind_t = const.tile([64, 392], bf16, name="ind", tag="ind")
        dma(out=ind_t, in_=ind_d)
        w9_t = const.tile([128, 8, 9], f32, name="w9", tag="w9")
        dma(out=w9_t, in_=w9_d)
        id_t = const.tile([128, 128], bf16, name="id", tag="id")
        dma(out=id_t, in_=id_d)
        # PE-conv consts: tap diagonals for chtiles >= CONV_DVE, B2^T, ind56
        dg_t = {}
        for ct in range(CONV_DVE, 8):
            dg_t[ct] = const.tile([128, 9, 128], bf16, name=f"dg{ct}",
                                  tag=f"dg{ct}")
            dma(out=dg_t[ct], in_=bass.AP(
                tensor=dg_d.tensor, offset=ct * 9 * 128 * 128,
                ap=[[128, 128], [128 * 128, 9], [1, 128]]))
        b2T_t = const.tile([64, 1024], bf16, name="b2T", tag="b2T")
        dma(out=b2T_t, in_=b2T_d)
        i64_t = const.tile([64, 512], bf16, name="i64", tag="i64")
        dma(out=i64_t, in_=i64_d)

        # ---------------- persistent tiles ----------------
        xT = [pers.tile([128, NT], bf16, name=f"xT{k}", tag=f"xT{k}") for k in range(3)]
        qcm = [pers.tile([128, NT], bf16, name=f"q{t}", tag=f"q{t}") for t in range(2)]
        # k channel-major in 64-slot pitch so logits' stationary slices are
        # contiguous and logit rows land on the 64-slot m-dim
        kcm = [pers.tile([128, NI * 64], bf16, name=f"k{t}", tag=f"k{t}")
               for t in range(2)]
        # v channel-major in 64-slot layout [8i x 8y x 8x] with 16-col
        # guards each end: [128, 16+512+16]. Pad slots and guards stay 0.
        vcm = [[pers.tile([128, 544], bf16, name=f"vc{c}_{s}", tag=f"vc{c}_{s}")
                for s in range(2)] for c in range(8)]
        vtok = [pers.tile([128, DH], bf16, name=f"vt{s}", tag=f"vt{s}") for s in range(8)]
        Ls = [[pers.tile([128, 392], bf16, name=f"Ls{j}_{s}", tag=f"Ls{j}_{s}") for s in range(2)]
              for j in range(4)]
        a2lo = [[pers.tile([128, 512], bf16, name=f"a2l{j}_{s}", tag=f"a2l{j}_{s}") for s in range(2)]
                for j in range(4)]
        a2hi = [[pers.tile([128, 512], bf16, name=f"a2h{j}_{s}", tag=f"a2h{j}_{s}") for s in range(2)]
                for j in range(4)]
        r_sb = [pers.tile([128, 392], bf16, name=f"rsb{s}", tag=f"rsb{s}") for s in range(2)]
        out_cm = [pers.tile([128, NT], bf16, name=f"oc{m}", tag=f"oc{m}") for m in range(3)]

        # zero-init: v tiles (pad slots/guards must be 0) and kcm (pad
        # slots feed logits stationary; must be 0, and never NaN)
        for c in range(8):
            for s in range(2):
                nc.gpsimd.memset(vcm[c][s], 0.0)
        for t in range(2):
            nc.gpsimd.memset(kcm[t], 0.0)
        for j in range(4):
            for s in range(2):
                nc.gpsimd.memset(a2lo[j][s], 0.0)
                nc.gpsimd.memset(a2hi[j][s], 0.0)

        # ---------------- x load + transpose (JIT per wave) ----------------
        def load_x(t0, t1):
            for tt in range(t0, min(t1, NTT)):
                rows = min(128, NT - tt * 128)
                xt_in = xin.tile([128, DIM], bf16, name="xin", tag="xin",
                                 bufs=4)
                dma(out=xt_in[0:rows, :], in_=x_d[tt * 128: tt * 128 + rows, :])
                for kc in range(3):
                    tp = ps.tile([128, 512], bf16, name="ps", tag="ps")
                    nc.tensor.transpose(tp[0:128, 0:rows],
                                        xt_in[0:rows, kc * 128:(kc + 1) * 128],
                                        id_t[0:rows, 0:rows])
                    nc.vector.tensor_copy(xT[kc][:, tt * 128: tt * 128 + rows],
                                          tp[0:128, 0:rows])

        def t_hi(w):
            return (392 * (w + 1) + 127) // 128

        load_x(0, t_hi(0))

        # ---------------- software-pipelined wave loop ----------------
        # Per-engine streams are ordered so that group w-1's back half
        # (attnV/conv/relu/proj) interleaves with group w's front half,
        # keeping every sequencer fed across cross-engine handoffs.

        def qkv(g):
            sl, c0 = g % 2, g * 392  # noqa: g used for kcm offset
            for mt in range(12):
                qp = ps.tile([128, 512], f32, name="ps", tag="ps")
                for kt in range(3):
                    if mt < 4:
                        w_ = wqk_t[kt][:, mt * 128:(mt + 1) * 128]
                    else:
                        w_ = wv_t[kt][:, (mt - 4) * 128:(mt - 3) * 128]
                    nc.tensor.matmul(qp[:, 0:392], w_,
                                     xT[kt][:, c0:c0 + 392],
                                     start=(kt == 0), stop=(kt == 2))
                sv = qp[:, 0:392].rearrange("p (i y x) -> p i y x",
                                            i=8, y=7, x=7)
                if mt < 2:
                    dst = qcm[mt % 2]
                    nc.scalar.activation(dst[:, c0:c0 + 392], qp[:, 0:392],
                                         AF.Identity,
                                         bias=bqk_t[:, mt:mt + 1])
                elif mt < 4:
                    dv = kcm[mt % 2][:, g * 512: (g + 1) * 512]
                    dv = dv.rearrange("p (i y x) -> p i y x", i=8, y=8, x=8)
                    dv = dv[:, :, 0:7, 0:7]
                    nc.scalar.activation(dv, sv, AF.Identity,
                                         bias=bqk_t[:, mt:mt + 1])
                else:
                    ct = mt - 4
                    dv = vcm[ct][sl][:, 16:528]
                    dv = dv.rearrange("p (i y x) -> p i y x", i=8, y=8, x=8)
                    dv = dv[:, :, 0:7, 0:7]
                    nc.scalar.activation(dv, sv, AF.Copy)

        def tr_vtok(g):
            # v token-major via PE transposes of channel-major v
            sl = g % 2
            for pr in range(4):
                p = 4 * g + pr
                vp = [ps.tile([128, 512], bf16, name="ps", tag="ps")
                      for _ in range(2)]
                for ig2 in range(2):
                    i = 2 * pr + ig2
                    for ct in range(8):
                        nh, cc = ct // 4, ct % 4
                        mv = vcm[ct][sl][:, 16 + i * 64: 16 + (i + 1) * 64]
                        nc.tensor.transpose(
                            vp[nh][64 * ig2: 64 * ig2 + 64,
                                   cc * 128:(cc + 1) * 128],
                            mv, id_t[0:128, 0:128])
                for nh in range(2):
                    if nh == 0:
                        nc.vector.tensor_copy(
                            vtok[p % 8][:, nh * 512:(nh + 1) * 512],
                            vp[nh])
                    else:
                        nc.scalar.activation(
                            vtok[p % 8][:, nh * 512:(nh + 1) * 512],
                            vp[nh], AF.Copy)

        def conv_dve(g):
            # DVE-path depthwise conv: B2-seeded bf16 acc, 9 TSP taps
            sl = g % 2
            acc_t = []
            for ct in range(CONV_DVE):
                acc = accp.tile([128, 512], bf16, name=f"acc{ct}",
                                tag=f"acc{ct}", bufs=2)
                dma(out=acc, in_=bass.AP(tensor=b2_d.tensor,
                                         offset=ct * 128 * 64,
                                         ap=[[64, 128], [0, 8], [1, 64]]))
                src = vcm[ct][sl]
                for tap in range(9):
                    dy, dx = tap // 3 - 1, tap % 3 - 1
                    b = 16 + 8 * dy + dx
                    nc.vector.scalar_tensor_tensor(
                        out=acc, in0=src[:, b: b + 512],
                        scalar=w9_t[:, ct, tap:tap + 1],
                        in1=acc, op0=ALU.mult, op1=ALU.add)
                acc_t.append(acc)
            return acc_t

        def logits(g):
            sl = g % 2
            Lp = [ps.tile([128, 512], f32, name="ps", tag="ps")
                  for _ in range(4)]
            for ig in range(8):
                i = g * 8 + ig
                for h in range(H):
                    j, hh = h // 2, h % 2
                    t4, row = h // 4, (h % 4) * 32
                    nc.tensor.matmul(
                        Lp[j][64 * hh: 64 * hh + 64, ig * N:(ig + 1) * N],
                        kcm[t4][row:row + 32, i * 64:(i + 1) * 64],
                        qcm[t4][row:row + 32, i * N:(i + 1) * N],
                        start=True, stop=True,
                        tile_position=(row, 64 * hh))
            for j in range(4):
                nc.scalar.activation(Ls[j][sl], Lp[j][:, 0:392], AF.Copy)

        def th1_exp(g):
            sl = g % 2
            E = []
            L2p = [ps.tile([128, 512], f32, name="ps", tag="ps")
                   for _ in range(4)]
            for jo in range(4):
                for ji in range(4):
                    nc.tensor.matmul(L2p[jo][:, 0:392],
                                     w1_t[:, jo * 4 + ji, :],
                                     Ls[ji][sl],
                                     start=(ji == 0), stop=False)
                nc.tensor.matmul(L2p[jo][:, 0:392],
                                 bT_t[0:N, jo, :],
                                 ind_t[0:N, :],
                                 start=False, stop=True)
            for jo in range(4):
                e = mid.tile([128, 392], bf16, name="E", tag="E")
                nc.scalar.activation(e, L2p[jo][:, 0:392], AF.Exp)
                E.append(e)
            return E

        def smax(g, E):
            sl = g % 2
            csp = ps.tile([128, 512], f32, name="ps", tag="ps")
            for j in range(4):
                nc.tensor.matmul(csp[32 * j: 32 * j + 2, 0:392], sel_t, E[j],
                                 start=True, stop=True,
                                 tile_position=(0, 32 * j))
            with nc.allow_low_precision(reason="bf16 1/r within tolerance"):
                for j in range(4):
                    nc.vector.reciprocal(r_sb[sl][32 * j: 32 * j + 2, :],
                                         csp[32 * j: 32 * j + 2, 0:392])

        def norm(g, E):
            sl = g % 2
            A = []
            for j in range(4):
                rp = ps.tile([128, 512], f32, name="ps", tag="ps")
                nc.tensor.matmul(rp[:, 0:392], dlt_t[32 * j: 32 * j + 2, :],
                                 r_sb[sl][32 * j: 32 * j + 2, :],
                                 start=True, stop=True,
                                 tile_position=(32 * j, 0))
                a = mid.tile([128, 392], bf16, name="A", tag="A")
                nc.vector.tensor_mul(a, E[j], rp[:, 0:392])
                A.append(a)
            return A

        def th2(g, A):
            sl = g % 2
            A2p = [ps.tile([128, 512], f32, name="ps", tag="ps")
                   for _ in range(4)]
            for jo in range(4):
                for ji in range(4):
                    nc.tensor.matmul(A2p[jo][:, 0:392],
                                     w2_t[:, jo * 4 + ji, :],
                                     A[ji],
                                     start=(ji == 0), stop=(ji == 3))
            for jo in range(4):
                sv = A2p[jo][:, 0:392].rearrange("p (i y x) -> p i y x",
                                                 y=7, x=7)
                dv = a2lo[jo][sl].rearrange("p (i y x) -> p i y x",
                                            y=8, x=8)[:, :, 0:7, 0:7]
                nc.scalar.activation(dv, sv, AF.Copy)
                dh = a2hi[jo][sl].rearrange("p (i y x) -> p i y x",
                                            y=8, x=8)[:, :, 0:7, 0:7]
                nc.vector.tensor_copy(dh[0:64], sv[64:128])
                nc.vector.tensor_copy(dh[64:128], sv[0:64])

        def attnv(g, acc_t, cts, relu_t):
            sl = g % 2
            for ct in cts:
                jo, hh = ct // 2, ct % 2
                if ct < CONV_DVE:
                    # DVE path: attnV into parity psums (64-slot pitch),
                    # v_local added on DVE from the TSP acc, relu to dense rl.
                    op2 = [ps.tile([128, 512], f32, name="ps", tag="ps")
                           for _ in range(2)]
                    for ig in range(8):
                        i = g * 8 + ig
                        pp, i4 = ig % 2, ig // 2
                        a2 = (a2lo if hh == pp else a2hi)[jo][sl]
                        nc.tensor.matmul(
                            op2[pp][:, i4 * 64: i4 * 64 + 64],
                            vtok[(i // 2) % 8][64 * pp: 64 * pp + 64,
                                               ct * 128:(ct + 1) * 128],
                            a2[64 * pp: 64 * pp + 64,
                               ig * 64:(ig + 1) * 64],
                            start=True, stop=True)
                    tmp = mid.tile([128, 392], bf16, name="tmp", tag="tmp",
                                   bufs=3)
                    accv = acc_t[ct].rearrange(
                        "p (i4 two y x) -> p i4 two y x", two=2, y=8, x=8)
                    tmpv = tmp.rearrange(
                        "p (i4 two y x) -> p i4 two y x", two=2, y=7, x=7)
                    for pp in range(2):
                        nc.vector.tensor_add(
                            tmpv[:, :, pp],
                            op2[pp][:, 0:256].rearrange(
                                "p (i y x) -> p i y x", y=8, x=8)[:, :, 0:7,
                                                                  0:7],
                            accv[:, :, pp, 0:7, 0:7])
                    rl = mid.tile([128, 392], bf16, name="rl", tag="rl",
                                  bufs=16)
                    nc.scalar.activation(rl, tmp, AF.Relu)
                    relu_t[ct] = rl
                else:
                    # PE path: conv as matmuls — B2 seed + 9 full-span diag
                    # taps chained in one psum; attnV into two parity psums
                    # (even/odd images use different PE row tiles; separate
                    # banks avoid concurrent same-bank writes). DVE combines,
                    # ACT relu compacts 64-slot to dense rl.
                    opc = ps.tile([128, 512], f32, name="ps", tag="ps")
                    nc.tensor.matmul(opc,
                                     b2T_t[:, ct * 128:(ct + 1) * 128],
                                     i64_t, start=True, stop=False)
                    srcv = vcm[ct][sl]
                    for tap in range(9):
                        dy, dx = tap // 3 - 1, tap % 3 - 1
                        b = 16 + 8 * dy + dx
                        nc.tensor.matmul(opc, dg_t[ct][:, tap, :],
                                         srcv[:, b: b + 512],
                                         start=False, stop=(tap == 8))
                    opa = [ps.tile([128, 512], f32, name="ps", tag="ps")
                           for _ in range(2)]
                    for ig in range(8):
                        i = g * 8 + ig
                        pp, i4 = ig % 2, ig // 2
                        a2 = (a2lo if hh == pp else a2hi)[jo][sl]
                        nc.tensor.matmul(
                            opa[pp][:, i4 * 64: i4 * 64 + 64],
                            vtok[(i // 2) % 8][64 * pp: 64 * pp + 64,
                                               ct * 128:(ct + 1) * 128],
                            a2[64 * pp: 64 * pp + 64,
                               ig * 64:(ig + 1) * 64],
                            start=True, stop=True)
                    csb = mid.tile([128, 512], bf16, name="cs8", tag="cs8",
                                   bufs=3)
                    nc.scalar.activation(csb, opc, AF.Copy)
                    tmp = mid.tile([128, 512], bf16, name="tp8", tag="tp8",
                                   bufs=3)
                    tv = tmp.rearrange("p (i4 two s) -> p i4 two s",
                                       two=2, s=64)
                    cv = csb.rearrange("p (i4 two s) -> p i4 two s",
                                       two=2, s=64)
                    for pp in range(2):
                        nc.vector.tensor_add(
                            tv[:, :, pp],
                            opa[pp][:, 0:256].rearrange(
                                "p (i4 s) -> p i4 s", s=64),
                            cv[:, :, pp])
                    rl = mid.tile([128, 392], bf16, name="rl", tag="rl",
                                  bufs=16)
                    iv = tmp.rearrange("p (i y x) -> p i y x",
                                       i=8, y=8, x=8)[:, :, 0:7, 0:7]
                    ov = rl.rearrange("p (i y x) -> p i y x", i=8, y=7, x=7)
                    nc.scalar.activation(ov, iv, AF.Relu)
                    relu_t[ct] = rl

        def proj(g, relu_t):
            c0 = g * 392
            for mt in range(3):
                pp_ = ps.tile([128, 512], f32, name="ps", tag="ps")
                for kt in range(8):
                    mv = relu_t[kt]
                    nc.tensor.matmul(pp_[:, 0:392],
                                     wp_t[kt][:, mt * 128:(mt + 1) * 128],
                                     mv,
                                     start=(kt == 0), stop=(kt == 7))
                nc.scalar.activation(out_cm[mt][:, c0:c0 + 392],
                                     pp_[:, 0:392], AF.Identity,
                                     bias=bp_t[:, mt:mt + 1])

        def store_out(t0, t1):
            for tt in range(t0, min(t1, NTT)):
                rows = min(128, NT - tt * 128)
                st = stg.tile([128, DIM], f32, name="st", tag="st", bufs=3)
                for mt in range(3):
                    tp = ps.tile([128, 512], bf16, name="ps", tag="ps")
                    nc.tensor.transpose(tp[0:rows, 0:128],
                                        out_cm[mt][:, tt * 128: tt * 128 + rows],
                                        id_t[0:128, 0:128])
                    if mt == 1:
                        nc.vector.tensor_copy(st[0:rows, mt * 128:(mt + 1) * 128],
                                              tp[0:rows, 0:128])
                    else:
                        nc.scalar.activation(st[0:rows, mt * 128:(mt + 1) * 128],
                                             tp[0:rows, 0:128], AF.Copy)
                dma(out=out_d[tt * 128: tt * 128 + rows, :], in_=st[0:rows, :])

        S = stage
        zr = stg.tile([128, DIM], f32, name="st", tag="st", bufs=3)
        nc.vector.memset(zr, 0.0)
        if S < 11:
            for tt in range(NTT):
                rows = min(128, NT - tt * 128)
                dma(out=out_d[tt * 128: tt * 128 + rows, :], in_=zr[0:rows, :])
        acc_prev = None
        relu_prev = None
        for w in range(NG + 1):
            bk = w >= 1 and S >= 9   # emit back half of wave w-1
            if bk:
                relu_cur = [None] * 8
            if w < NG:
                load_x(t_hi(w), t_hi(w + 1))   # prefetch wave w+1's x tiles
                if S >= 2:
                    qkv(w)
            if bk:
                attnv(w - 1, acc_prev, range(0, min(5, 8 if S >= 10 else CONV_DVE)), relu_cur)
            if w < NG:
                if S >= 3:
                    tr_vtok(w)
                if S >= 4:
                    logits(w)
                if S >= 5:
                    E = th1_exp(w)
                if bk and S >= 10:
                    attnv(w - 1, acc_prev, [5], relu_cur)
                if S >= 6:
                    smax(w, E)
                if bk and S >= 10:
                    attnv(w - 1, acc_prev, [6], relu_cur)
                if S >= 6:
                    A = norm(w, E)
                if bk and S >= 10:
                    attnv(w - 1, acc_prev, [7], relu_cur)
                if S >= 7:
                    th2(w, A)
            elif bk and S >= 10:
                attnv(w - 1, acc_prev, range(5, 8), relu_cur)
            if bk and S >= 11:
                proj(w - 1, relu_cur)
                store_out((392 * (w - 1)) // 128, (392 * w) // 128)
            if w < NG and S >= 8:
                acc_prev = conv_dve(w)
        if S >= 11:
            store_out((392 * (NG - 1)) // 128, NTT)

    nc.compile()
    return nc


_CACHE = {}


def _get_program(n_imgs):
    if n_imgs not in _CACHE:
        _CACHE[n_imgs] = build_program(n_imgs)
    return _CACHE[n_imgs]


def make_in_maps(inputs, n_cores=NCORES):
    """Host prep: shard x, build replicated constants."""
    consts = _host_consts(inputs)
    x = np.asarray(inputs['x'], np.float32)
    B = x.shape[0]
    ni = B // n_cores
    x = x.reshape(B, N, DIM)
    in_maps = []
    for c in range(n_cores):
        m = dict(consts)
        m['x'] = x[c * ni:(c + 1) * ni].reshape(ni * N, DIM).astype(_BF16)
        in_maps.append(m)
    return in_maps, ni


def kernel(**inputs):
    from concourse import bass_utils
    in_maps, ni = make_in_maps(inputs)
    nc = _get_program(ni)
    res = bass_utils.run_bass_kernel_spmd(
        nc, in_maps, core_ids=list(range(NCORES)))
    B = np.asarray(inputs['x']).shape[0]
    out = np.concatenate([r['out'] for r in res.results], axis=0)
    return out.reshape(B, R, R, DIM).astype(np.float32)

